# revision 18
# baseline (speedup 1.0000x reference)
"""Trainium2 Bass kernel for nn_Diff_Label01_Loss (masked cosine-similarity loss).

Contract: kernel(labels, datas) takes FULL inputs (labels [8192,2] f32,
datas [8192,4096] f32) and returns (total_loss, sim_loss, differ_loss)
matching the reference. Internally shards the batch across 8 NeuronCores
(data-parallel), all-reduces the masked column sum on-device, and does the
final scalar arithmetic on host.

Math notes:
 - cos_i = (x_i . m) / (|x_i| |m|) is scale-invariant in m, so the device
   works with the raw masked sum s0 = sum_{label==0} x_i (never divides by
   n0) and the host folds 1/n0 and 1/|s0| into the final scalars.
 - datas is pre-cast to bf16 on host: halves HBM traffic (memory-bound
   kernel) and enables the DVE 2x perf mode. Validated numerically against
   the reference: worst-case relative error ~1.5e-4 (on differ_loss).
 - The all-reduce runs in bf16 and the per-row dots use exactly the
   all-reduced bf16 vector, so |m| (msq) is consistent with the numerators.

Implementation is raw Bass (explicit engine blocks + semaphores): the
installed walrus rejects TileContext's multi-wait Drain and the raw-ISA
fused ops (tensor_tensor_reduce, partition_broadcast), so the kernel sticks
to activation+accum / tensor_mul / tensor_reduce and hand-rolled sync.

Engine plan per core (shard = 1024 rows x 4096, resident in SBUF as bf16):
  ingest: x tiles split across both HWDGE rings (SP: even, ACT: odd)
  pass 1 (overlapped with ingest):
    PE : masked column-sum via matmul (mask column stationary) -> PSUM[8,512]
    DVE: squares into rotating dumps
    ACT: s0 PSUM->SBUF bf16 cast FIRST (unblocks the all-reduce at the
         earliest moment), then Copy+accum reduces for tiles 0..4;
         DVE reduces tiles 5..7 itself
  all-reduce: DMA s0 -> collective(bf16 add, 8KB) -> DMA back [1,4096]
  broadcast: PE K=1 matmul (ones x s0) -> PSUM ping-pong -> DVE copies to mb
  pass 2: DVE multiplies tiles by mb; ACT Copy+accum reduces tiles 0..6,
    DVE reduces tile 7
  finish: sqrt/reciprocal/abs + mask-weighted sums -> out[128, 0:2],
    msq (from the post-all-reduce vector) -> out[0, 2]; host combines.
"""

import contextlib

import numpy as np

B = 8192
D = 4096
P = 128
NCORES = 8
ROWS = B // NCORES          # 1024 rows per core
T = ROWS // P               # 8 row-tiles of 128 rows per core
MMCOL = 512                 # matmul moving free-dim limit
NCH = D // MMCOL            # 8 column chunks
EPS = 1e-8

ACT_R1 = 5                  # pass-1 reduces: tiles 0..4 on ACT, 5..7 on DVE
ACT_R2 = 7                  # pass-2 reduces: tiles 0..6 on ACT, 7 on DVE
NTMP = 4                    # rotating dump buffers per pass


def _build_program():
    import concourse.bass as bass
    import concourse.mybir as mybir

    f32 = mybir.dt.float32
    bf16 = mybir.dt.bfloat16
    AOP = mybir.AluOpType
    AF = mybir.ActivationFunctionType
    AX = mybir.AxisListType

    nc = bass.Bass(trn_type="TRN2", num_devices=NCORES)

    xb = nc.dram_tensor("xb", [ROWS, D], bf16, kind="ExternalInput")
    m0b = nc.dram_tensor("m0b", [P, T], bf16, kind="ExternalInput")
    w0 = nc.dram_tensor("w0", [P, T], f32, kind="ExternalInput")
    w1 = nc.dram_tensor("w1", [P, T], f32, kind="ExternalInput")
    out = nc.dram_tensor("out", [P, 4], f32, kind="ExternalOutput")

    cc_in = nc.dram_tensor("cc_in", [1, D], bf16)
    cc_out = nc.dram_tensor("cc_out", [1, D], bf16, addr_space="Shared")

    ctx = contextlib.ExitStack()

    def sb(name, shape, dt):
        return ctx.enter_context(nc.sbuf_tensor(name, shape, dt))

    x_all = sb("x_all", [P, T * D], bf16)
    tmpk = [sb(f"tmpk{i}", [P, D], bf16) for i in range(NTMP)]   # pass-1 dumps
    tmp2 = [sb(f"tmp2_{i}", [P, D], bf16) for i in range(NTMP)]  # pass-2 dumps
    mb = sb("mb", [P, D], bf16)
    m0s = sb("m0s", [P, T], bf16)
    w0s = sb("w0s", [P, T], f32)
    w1s = sb("w1s", [P, T], f32)
    normsq = sb("normsq", [P, T], f32)
    numer = sb("numer", [P, T], f32)
    norm = sb("norm", [P, T], f32)
    inv = sb("inv", [P, T], f32)
    absn = sb("absn", [P, T], f32)
    q = sb("q", [P, T], f32)
    u0 = sb("u0", [P, T], f32)
    u1 = sb("u1", [P, T], f32)
    fin = sb("fin", [P, 2], f32)
    s0bf = sb("s0bf", [1, D], bf16)
    s0post = sb("s0post", [1, D], bf16)
    jnk = sb("jnk", [1, 4], f32)
    msq1 = sb("msq1", [1, 1], f32)

    psum_s0 = ctx.enter_context(nc.psum_tensor("psum_s0", [1, D], f32))

    def sem(name):
        return ctx.enter_context(nc.semaphore(name))

    dx = [sem(f"dx{t}") for t in range(T)]
    sm0 = sem("sm0")
    smw = sem("smw")          # w0 + w1 + onesr
    s_pe = sem("s_pe")
    s_s0 = sem("s_s0")
    s_ccin = sem("s_ccin")
    s_cc = sem("s_cc")
    s_mb = sem("s_mb")        # mb broadcast DMA done
    s_cc2 = sem("s_cc2")      # s0post DMA done
    s_m1 = sem("s_m1")
    s_r1 = sem("s_r1")
    s_nv = sem("s_nv")        # DVE pass-1 reduces
    s_sqrt = sem("s_sqrt")
    s_si = sem("s_si")
    s_m2 = sem("s_m2")
    s_r2 = sem("s_r2")
    s_nv2 = sem("s_nv2")
    s_abs = sem("s_abs")
    s_sq = sem("s_sq")
    s_u = sem("s_u")
    s_fin = sem("s_fin")
    s_msq = sem("s_msq")
    s_outd = sem("s_outd")

    xr = xb.rearrange("(t p) d -> t p d", p=P)

    def x_tile(t):
        return x_all[:, t * D : (t + 1) * D]

    def blk(buf, c):
        return buf[:, c * MMCOL : (c + 1) * MMCOL]

    with nc.Block() as block:

        @block.sync
        def _(sync):
            for t in range(0, T, 2):
                sync.dma_start(x_tile(t), xr[t]).then_inc(dx[t], 16)
            sync.dma_start(w0s[:], w0[:]).then_inc(smw, 16)
            sync.dma_start(w1s[:], w1[:]).then_inc(smw, 16)
            sync.wait_ge(s_s0, 1)
            sync.dma_start(cc_in[:], s0bf[:]).then_inc(s_ccin, 16)
            sync.wait_ge(s_cc, 1)
            sync.dma_start(s0post[:], cc_out[:]).then_inc(s_cc2, 16)
            sync.dma_start(mb[:], cc_out[0:1, :].to_broadcast((P, D))).then_inc(
                s_mb, 16
            )
            sync.wait_ge(s_fin, 2)
            sync.dma_start(out[:, 0:2], fin[:]).then_inc(s_outd, 16)
            sync.wait_ge(s_msq, 1)
            sync.dma_start(out[0:1, 2:3], msq1[:]).then_inc(s_outd, 16)
            sync.wait_ge(s_outd, 32)

        @block.tensor
        def _(pe):
            pe.wait_ge(sm0, 16)
            for t in range(T):
                pe.wait_ge(dx[t], 16)
                for c in range(NCH):
                    mm = nc.tensor.matmul(
                        blk(psum_s0, c),
                        m0s[:, t : t + 1],
                        blk(x_tile(t), c),
                        start=(t == 0),
                        stop=(t == T - 1),
                    )
                    if t == T - 1 and c == NCH - 1:
                        mm.then_inc(s_pe, 1)

        @block.scalar
        def _(sc):
            # mask + odd x tiles ingest on the ACT HWDGE ring (parallel with SP)
            sc.dma_start(m0s[:], m0b[:]).then_inc(sm0, 16)
            for t in range(1, T, 2):
                sc.dma_start(x_tile(t), xr[t]).then_inc(dx[t], 16)
            # preload the activation tables (Copy/Sqrt/Abs) while ingest runs
            sc.wait_ge(sm0, 16)
            sc.activation(jnk[0:1, 0:1], m0s[0:1, 0:1], AF.Copy, accum_out=jnk[0:1, 1:2])
            sc.sqrt(jnk[0:1, 2:3], m0s[0:1, 0:1])
            sc.activation(jnk[0:1, 3:4], m0s[0:1, 0:1], AF.Abs)
            # s0 PSUM -> SBUF bf16 as soon as PE finishes: all-reduce leaves early
            sc.wait_ge(s_pe, 1)
            sc.copy(s0bf[:], psum_s0[:]).then_inc(s_s0, 1)
            # pass-1 reduces for tiles 0..ACT_R1-1 (dump in place)
            for t in range(ACT_R1):
                sc.wait_ge(s_m1, t + 1)
                b = tmpk[t % NTMP][:]
                sc.activation(
                    b, b, AF.Copy, accum_out=normsq[:, t : t + 1]
                ).then_inc(s_r1, 1)
            # per-row norms
            sc.wait_ge(s_r1, ACT_R1)
            sc.wait_ge(s_nv, T - ACT_R1)
            sc.sqrt(norm[:], normsq[:]).then_inc(s_sqrt, 1)
            # |s0|^2 of the post-all-reduce vector (same values pass 2 uses);
            # runs during the mb broadcast DMA, off the critical tail
            sc.wait_ge(s_cc2, 16)
            sc.activation(
                tmpk[0][0:1, :], s0post[:], AF.Square, accum_out=msq1[:]
            ).then_inc(s_msq, 1)
            # pass-2 reduces for tiles 0..ACT_R2-1
            for t in range(ACT_R2):
                sc.wait_ge(s_m2, t + 1)
                b = tmp2[t % NTMP][:]
                sc.activation(
                    b, b, AF.Copy, accum_out=numer[:, t : t + 1]
                ).then_inc(s_r2, 1)
            # |numer| once every reduce is complete
            sc.wait_ge(s_r2, ACT_R2)
            sc.wait_ge(s_nv2, T - ACT_R2)
            sc.activation(absn[:], numer[:], AF.Abs).then_inc(s_abs, 1)
            # mask-weighted partition sums
            sc.wait_ge(s_u, 1)
            sc.activation(
                u0[:], u0[:], AF.Copy, accum_out=fin[:, 0:1]
            ).then_inc(s_fin, 1)
            sc.wait_ge(s_u, 2)
            sc.activation(
                u1[:], u1[:], AF.Copy, accum_out=fin[:, 1:2]
            ).then_inc(s_fin, 1)

        @block.vector
        def _(ve):
            # pass-1 squares
            for t in range(T):
                ve.wait_ge(dx[t], 16)
                if t >= NTMP:
                    ve.wait_ge(s_r1, t - NTMP + 1)
                nc.vector.tensor_mul(
                    tmpk[t % NTMP][:], x_tile(t), x_tile(t)
                ).then_inc(s_m1, 1)
            # pass-1 reduces for tiles ACT_R1..7 (own muls drained: s_m1 == T)
            ve.wait_ge(s_m1, T)
            for t in range(ACT_R1, T):
                nc.vector.tensor_reduce(
                    out=normsq[:, t : t + 1], in_=tmpk[t % NTMP][:],
                    axis=AX.X, op=AOP.add,
                ).then_inc(s_nv, 1)
            # pass 2: per-row dot = x .* mb
            ve.wait_ge(s_mb, 16)
            for t in range(T):
                if t >= NTMP:
                    ve.wait_ge(s_r2, t - NTMP + 1)
                nc.vector.tensor_mul(
                    tmp2[t % NTMP][:], x_tile(t), mb[:]
                ).then_inc(s_m2, 1)
            ve.wait_ge(s_m2, T)
            for t in range(ACT_R2, T):
                nc.vector.tensor_reduce(
                    out=numer[:, t : t + 1], in_=tmp2[t % NTMP][:],
                    axis=AX.X, op=AOP.add,
                ).then_inc(s_nv2, 1)
            # tail: q = |numer| / |x| and the mask-weighted products
            ve.wait_ge(s_sqrt, 1)
            nc.vector.reciprocal(inv[:], norm[:]).then_inc(s_si, 1)
            ve.wait_ge(s_abs, 1)
            ve.wait_ge(s_si, 1)
            ve.wait_ge(smw, 32)
            nc.vector.tensor_mul(q[:], absn[:], inv[:]).then_inc(s_sq, 1)
            ve.wait_ge(s_sq, 1)
            nc.vector.tensor_mul(u0[:], q[:], w0s[:]).then_inc(s_u, 1)
            nc.vector.tensor_mul(u1[:], q[:], w1s[:]).then_inc(s_u, 1)

        @block.gpsimd
        def _(gp):
            gp.wait_ge(s_ccin, 16)
            gp.collective_compute(
                "AllReduce",
                AOP.add,
                replica_groups=[list(range(NCORES))],
                ins=[cc_in[:]],
                outs=[cc_out[:]],
            ).then_inc(s_cc, 1)

    ctx.close()
    return nc


_PROGRAM = None
LAST_RESULT = None  # BassKernelResults of the most recent run (for profiling harnesses)


def _host_inputs(labels, datas):
    import ml_dtypes

    labels = np.asarray(labels, dtype=np.float32)
    datas = np.asarray(datas, dtype=np.float32)

    # argmax(labels, axis=1) == 0  <=>  labels[:,0] >= labels[:,1] (first max wins)
    mask0 = (labels[:, 0] >= labels[:, 1]).astype(np.float32)
    mask1 = np.float32(1.0) - mask0
    n0 = float(mask0.sum())
    n1 = float(mask1.sum())

    xbf = datas.astype(ml_dtypes.bfloat16)

    in_maps = []
    for c in range(NCORES):
        rows = slice(c * ROWS, (c + 1) * ROWS)
        # [ROWS] -> [P, T] with [p, t] = shard[t*P + p]
        m0c = mask0[rows].reshape(T, P).T.copy()
        m1c = mask1[rows].reshape(T, P).T.copy()
        in_maps.append(
            {
                "xb": np.ascontiguousarray(xbf[rows]),
                "m0b": m0c.astype(ml_dtypes.bfloat16),
                "w0": m0c,
                "w1": m1c,
            }
        )
    return in_maps, n0, n1


def _host_finish(outs, n0, n1):
    ssim = 0.0
    sdif = 0.0
    for c in range(NCORES):
        o = np.asarray(outs[c], dtype=np.float64)
        ssim += o[:, 0].sum()
        sdif += o[:, 1].sum()
    msq = float(np.asarray(outs[0])[0, 2])

    if n0 > 0.0:
        mnorm = max(np.sqrt(msq), EPS * n0)  # |s0|, with the reference's eps on |m|
        sim = 1.0 - ssim / (n0 * mnorm)
        dif = (sdif / (n1 * mnorm)) if n1 > 0.0 else 0.0
    else:
        sim = 0.0
        dif = 0.0

    sim = np.float32(sim)
    dif = np.float32(dif)
    return (np.float32(sim + dif), sim, dif)


def kernel(labels, datas):
    global _PROGRAM, LAST_RESULT
    from concourse.bass_utils import run_bass_kernel_spmd

    in_maps, n0, n1 = _host_inputs(labels, datas)
    if _PROGRAM is None:
        _PROGRAM = _build_program()
    res = run_bass_kernel_spmd(_PROGRAM, in_maps, list(range(NCORES)))
    LAST_RESULT = res
    outs = [res.results[c]["out"] for c in range(NCORES)]
    return _host_finish(outs, n0, n1)
